# revision 13
# baseline (speedup 1.0000x reference)
"""Causal multi-head attention (S=2048, B=2, H=16, D=128, fp32) on 8 trn2 cores.

Sharding: the 32 (batch, head) pairs are split 4-per-core (tensor parallel on
heads). Each core runs a flash-attention-style kernel in the "S^T layout",
processing key blocks two at a time:

  For a query chunk c (512 wide) and key-block pair (j0, j1) (128 wide each):
    S^T[k, q] = matmul: lhsT = K^T[d, k_j], rhs = Q^T[d, q_c]   (PE, fp16) x2
    P^T = exp(S^T)            (Q pre-scaled by 1/sqrt(D) on host)  (ACT)
      [selected pairs instead use a Schraudolph exp on DVE:
       P = bitcast_fp16(int16(S * 1024*log2e + 15315.5)), max rel err ~3%]
    causal wedge masked via affine_select on the diagonal 128x128s  (GpSimd)
    ctx^T[d, q_c] += matmul: lhsT = V[k_j, d], rhs = P^T           (PE) x2
    pacc[k, q]    += P^T halves                                    (DVE)
  End of chunk: one ones-like one-hot matmul folds pacc into row c of a
  per-head PSUM tile l4[4, 512] (l = softmax denominators); l4 is copied
  out once per head.

Engine balance: PE does only the two real GEMMs + 4 tiny folds/head; the
softmax row-sum accumulation lives on DVE; the causal mask on GpSimd; exp
mostly on ACT with a slice offloaded to DVE.

Host pre-transposes Q/K to [d, s] per head so no on-chip transposes are
needed anywhere, and does the final divide ctx/l (mathematically identical
to normalizing P before the V matmul).
"""

import sys

if "/opt/trn_rl_repo" not in sys.path:
    sys.path.insert(0, "/opt/trn_rl_repo")

import numpy as np

S, B, H, D = 2048, 2, 16, 128
N_CORES = 8
HPC = (B * H) // N_CORES  # head-slices per core = 4
QCH = 512  # query chunk width (one PSUM bank of fp32)
NCH = S // QCH  # 4 chunks
NKB = S // 128  # 16 key blocks
SCALE = 1.0 / float(np.sqrt(D))

QK_DTYPE = "float16"

# Schraudolph fp16 exp: exp(s) ~= bitcast_fp16(int16(s*EXP_A + EXP_B)).
EXP_A = 1024.0 / float(np.log(2.0))
EXP_B = 15315.5

# (c, pi) pairs whose exp runs on DVE instead of ACT (must be
# non-diagonal pairs, i.e. 2*pi+1 < 4*c).
DVE_EXP_PAIRS = {(2, 1), (3, 1), (3, 3)}
# (c, pi) pairs whose pacc accumulation runs on a GpSimd-owned
# accumulator (full-width non-diagonal pairs only).
GPS_PAIRS = set()

_compiled = None


def _build():
    import concourse.tile as tile
    from concourse import bacc, mybir

    f32 = mybir.dt.float32
    i16 = mybir.dt.int16
    qk_dt = getattr(mybir.dt, QK_DTYPE)

    nc = bacc.Bacc("TRN2", target_bir_lowering=False, debug=False)
    qT = nc.dram_tensor("qT", [HPC, D, S], qk_dt, kind="ExternalInput").ap()
    kT = nc.dram_tensor("kT", [HPC, D, S], qk_dt, kind="ExternalInput").ap()
    # v is pre-shuffled on host to [p, j*128+d] (partition-major) so the
    # load is a plain contiguous [128, 512]-per-chunk DMA
    v = nc.dram_tensor("v", [HPC, 128, NKB * 128], qk_dt, kind="ExternalInput").ap()
    out = nc.dram_tensor("out", [HPC, D, S], qk_dt, kind="ExternalOutput").ap()
    lsum = nc.dram_tensor("lsum", [HPC, NCH, QCH], f32, kind="ExternalOutput").ap()

    with tile.TileContext(nc) as tc:
        with (
            tc.tile_pool(name="const", bufs=1) as const_pool,
            tc.tile_pool(name="io", bufs=2) as io_pool,
            tc.tile_pool(name="p", bufs=6) as p_pool,
            tc.tile_pool(name="acc", bufs=2) as acc_pool,
            tc.tile_pool(name="o", bufs=3) as o_pool,
            tc.tile_pool(name="psum_s", bufs=3, space="PSUM") as psum_s,
            tc.tile_pool(name="psum_ctx", bufs=1, space="PSUM") as psum_ctx,
            tc.tile_pool(name="psum_l", bufs=1, space="PSUM") as psum_l,
        ):
            # one-hot stationaries: oh[:, 4c + c] == 1, used to fold chunk
            # c's pacc row-sum into partition c of l4
            oh = const_pool.tile([128, 4 * NCH], qk_dt)
            nc.vector.memset(oh[:], 0.0)
            for c in range(NCH):
                nc.vector.memset(oh[:, 5 * c : 5 * c + 1], 1.0)

            for h in range(HPC):
                # chunked loads so chunk-0 compute starts before the whole
                # head is resident
                qT_s = io_pool.tile([128, S], qk_dt, tag="qT_s")
                kT_s = io_pool.tile([128, S], qk_dt, tag="kT_s")
                v_s = io_pool.tile([128, NKB * 128], qk_dt, tag="v_s")
                for c in range(NCH):
                    sl = slice(c * QCH, (c + 1) * QCH)
                    if h == 0 and c == 0:
                        # split across queues for fast first-compute; issue
                        # order = first-needed order (k0, q all, v, rest of k)
                        nc.sync.dma_start(kT_s[:, 0:128], kT[h][:, 0:128])
                        for q in range(4):
                            s4 = slice(q * 128, (q + 1) * 128)
                            nc.sync.dma_start(qT_s[:, s4], qT[h][:, s4])
                        nc.sync.dma_start(kT_s[:, 128:256], kT[h][:, 128:256])
                        for q in range(2):
                            s4 = slice(q * 256, (q + 1) * 256)
                            nc.sync.dma_start(v_s[:, s4], v[h][:, s4])
                        nc.sync.dma_start(kT_s[:, 256:512], kT[h][:, 256:512])
                    else:
                        nc.sync.dma_start(kT_s[:, sl], kT[h][:, sl])
                        nc.sync.dma_start(qT_s[:, sl], qT[h][:, sl])
                        nc.sync.dma_start(v_s[:, sl], v[h][:, sl])

                chunk_order = (
                    list(range(NCH)) if h == 0 else list(range(NCH - 1, -1, -1))
                )
                l4 = psum_l.tile([NCH, QCH], f32, tag="l4")
                nfolds = NCH
                fold_idx = 0
                for ci, c in enumerate(chunk_order):
                    qmov = qT_s[:, c * QCH : (c + 1) * QCH]
                    ctx_c = psum_ctx.tile([128, QCH], f32, tag="ctx")
                    pacc = acc_pool.tile([128, 2 * QCH], qk_dt, tag="pacc")
                    npairs = 2 * c + 2
                    # diagonal pairs first: their exp->mask->BMM2 chain then
                    # overlaps the rest of the chunk instead of serializing
                    # at the chunk boundary
                    pair_order = [2 * c, 2 * c + 1] + list(range(2 * c))
                    for pii, pi in enumerate(pair_order):
                        j0, j1 = 2 * pi, 2 * pi + 1
                        off = [j - 4 * c for j in (j0, j1)]
                        # causal trim: for diagonal blocks (off >= 0) the
                        # first 128*off query columns are fully masked;
                        # skip them everywhere.
                        w = [128 * max(0, o) for o in off]
                        s2 = psum_s.tile([128, 2 * QCH], f32, tag="s2")
                        p2 = p_pool.tile([128, 2 * QCH], qk_dt, tag="p2")
                        for o, j in enumerate((j0, j1)):
                            nc.tensor.matmul(
                                s2[:, o * QCH + w[o] : (o + 1) * QCH],
                                kT_s[:, j * 128 : (j + 1) * 128],
                                qmov[:, w[o] :],
                                start=True,
                                stop=True,
                            )
                        if (c, pi) in DVE_EXP_PAIRS:
                            # non-diagonal pair: Schraudolph exp on DVE
                            nc.vector.tensor_scalar(
                                p2[:].bitcast(i16),
                                s2[:],
                                EXP_A,
                                EXP_B,
                                mybir.AluOpType.mult,
                                mybir.AluOpType.add,
                            )
                        elif off[0] == 2:
                            # diagonal pair (offsets 2,3): per-half exp on
                            # the exact surviving spans
                            nc.scalar.activation(
                                p2[:, 256:512],
                                s2[:, 256:512],
                                mybir.ActivationFunctionType.Exp,
                            )
                            nc.scalar.activation(
                                p2[:, QCH + 384 :],
                                s2[:, QCH + 384 :],
                                mybir.ActivationFunctionType.Exp,
                            )
                        else:
                            # non-diagonal pair, or diagonal pair (0,1):
                            # one 1024-wide exp ([512:640] of the (0,1)
                            # pair is stale; the affine_select below
                            # zero-fills it)
                            nc.scalar.activation(
                                p2[:],
                                s2[:],
                                mybir.ActivationFunctionType.Exp,
                            )
                        # causal wedge masks (keep where x' - p >= 0)
                        if off[0] == 0:
                            # both blocks' wedges (plus the h1 stale prefix
                            # [512:640]) in ONE select over a [p, o, x] view:
                            # keep iff x - 128*o - p >= 0; cols x in
                            # [256:512) of either half are always kept and
                            # excluded from the AP
                            sel = p2[:].rearrange("p (o x) -> p o x", o=2)[
                                :, :, 0:256
                            ]
                            nc.gpsimd.affine_select(
                                sel,
                                sel,
                                pattern=[[-128, 2], [1, 256]],
                                base=0,
                                channel_multiplier=-1,
                                compare_op=mybir.AluOpType.is_ge,
                                fill=0.0,
                            )
                        elif off[0] == 2:
                            nc.gpsimd.affine_select(
                                p2[:, 256:384],
                                p2[:, 256:384],
                                pattern=[[1, 128]],
                                base=0,
                                channel_multiplier=-1,
                                compare_op=mybir.AluOpType.is_ge,
                                fill=0.0,
                            )
                            nc.gpsimd.affine_select(
                                p2[:, QCH + 384 :],
                                p2[:, QCH + 384 :],
                                pattern=[[1, 128]],
                                base=0,
                                channel_multiplier=-1,
                                compare_op=mybir.AluOpType.is_ge,
                                fill=0.0,
                            )
                        for o, j in enumerate((j0, j1)):
                            nc.tensor.matmul(
                                ctx_c[:, w[o] :],
                                v_s[:, j * 128 : (j + 1) * 128],
                                p2[:, o * QCH + w[o] : (o + 1) * QCH],
                                start=(pii == 0 and o == 0),
                                stop=(pii == npairs - 1 and o == 1),
                                skip_group_check=True,
                            )
                        # accumulate P into pacc / paccg: pair (0,1)
                        # initializes pacc with a full copy (its masked
                        # regions are zero after the select); pair (2,3)
                        # adds only its surviving spans; non-diagonal pairs
                        # add full 1024-wide (one DVE op per pair), except
                        # GPS_PAIRS which accumulate on GpSimd
                        if pii == 0:
                            nc.vector.tensor_copy(pacc[:], p2[:])
                        elif off[0] == 2:
                            nc.vector.tensor_add(
                                pacc[:, 256:512],
                                pacc[:, 256:512],
                                p2[:, 256:512],
                            )
                            nc.vector.tensor_add(
                                pacc[:, QCH + 384 :],
                                pacc[:, QCH + 384 :],
                                p2[:, QCH + 384 :],
                            )
                        else:
                            # 512-wide halves: fp16 tensor_tensor only gets
                            # the 2x uop up to FD=512; 1024-wide runs 1x
                            nc.vector.tensor_add(
                                pacc[:, 0:QCH], pacc[:, 0:QCH], p2[:, 0:QCH]
                            )
                            nc.vector.tensor_add(
                                pacc[:, QCH:], pacc[:, QCH:], p2[:, QCH:]
                            )
                    # fold chunk row-sums into partition c of l4 (both
                    # halves of each accumulator)
                    # sum the two half-accumulators on DVE, then one fold
                    nc.vector.tensor_add(
                        pacc[:, 0:QCH], pacc[:, 0:QCH], pacc[:, QCH:]
                    )
                    nc.tensor.matmul(
                        l4[:],
                        oh[:, 4 * c : 4 * c + 4],
                        pacc[:, 0:QCH],
                        start=(fold_idx == 0),
                        stop=(fold_idx == nfolds - 1),
                        skip_group_check=True,
                    )
                    fold_idx += 1
                    o_t = o_pool.tile([128, QCH], qk_dt, tag="o")
                    # drain ctx in two halves on different engines: spreads
                    # load and frees the PSUM bank sooner
                    nc.scalar.copy(o_t[:, 0:256], ctx_c[:, 0:256])
                    nc.vector.tensor_copy(o_t[:, 256:], ctx_c[:, 256:])
                    nc.sync.dma_start(
                        out[h][:, c * QCH : (c + 1) * QCH], o_t[:]
                    )
                lo_t = o_pool.tile([NCH, QCH], f32, tag="lo")
                nc.vector.tensor_copy(lo_t[:], l4[:])
                nc.sync.dma_start(lsum[h], lo_t[:])

    nc.compile()
    return nc


def _get_compiled():
    global _compiled
    if _compiled is None:
        _compiled = _build()
    return _compiled


def _run(query_layer, key_layer, value_layer, attention_mask=None, trace=False):
    from concourse import bass_utils

    nc = _get_compiled()

    q = np.asarray(query_layer, dtype=np.float32)
    k = np.asarray(key_layer, dtype=np.float32)
    v = np.asarray(value_layer, dtype=np.float32)

    np_dt = np.float16 if QK_DTYPE == "float16" else np.float32

    # [S,B,H,D] -> [BH, D, S] for q/k, [BH, S, D] for v.
    # Fold the 1/sqrt(D) softmax scale into Q on the host.
    qT_all = np.ascontiguousarray(
        (q.transpose(1, 2, 3, 0).reshape(B * H, D, S) * np.float32(SCALE)).astype(
            np_dt
        )
    )
    kT_all = np.ascontiguousarray(
        k.transpose(1, 2, 3, 0).reshape(B * H, D, S).astype(np_dt)
    )
    # [S,B,H,D] -> [BH, S, D] -> partition-major [BH, p, j, d] with
    # s = 128*j + p, flattened to [BH, 128, NKB*128]
    v_all = np.ascontiguousarray(
        v.transpose(1, 2, 0, 3)
        .reshape(B * H, NKB, 128, D)
        .transpose(0, 2, 1, 3)
        .reshape(B * H, 128, NKB * 128)
        .astype(np_dt)
    )

    in_maps = [
        {
            "qT": qT_all[c * HPC : (c + 1) * HPC],
            "kT": kT_all[c * HPC : (c + 1) * HPC],
            "v": v_all[c * HPC : (c + 1) * HPC],
        }
        for c in range(N_CORES)
    ]
    res = bass_utils.run_bass_kernel_spmd(
        nc, in_maps, list(range(N_CORES)), trace=trace
    )

    ctxT = np.concatenate(
        [np.asarray(res.results[c]["out"], dtype=np.float32) for c in range(N_CORES)],
        axis=0,
    )  # [BH, D, S]
    l = np.concatenate(
        [res.results[c]["lsum"].reshape(HPC, S) for c in range(N_CORES)], axis=0
    )  # [BH, S]
    ctxT = ctxT / l[:, None, :]
    # [BH, D, S] -> [S, B, H*D]
    full = ctxT.reshape(B, H, D, S).transpose(3, 0, 1, 2).reshape(S, B, H * D)
    return np.ascontiguousarray(full.astype(np.float32)), res


def kernel(query_layer, key_layer, value_layer, attention_mask=None):
    out, _ = _run(query_layer, key_layer, value_layer, attention_mask)
    return out


# revision 14
# speedup vs baseline: 1.2014x; 1.2014x over previous
"""Causal multi-head attention (S=2048, B=2, H=16, D=128, fp32) on 8 trn2 cores.

Sharding: the 32 (batch, head) pairs are split 4-per-core (tensor parallel on
heads). Each core runs a flash-attention-style kernel in the "S^T layout",
processing key blocks two at a time:

  For a query chunk c (512 wide) and key-block pair (j0, j1) (128 wide each):
    S^T[k, q] = matmul: lhsT = K^T[d, k_j], rhs = Q^T[d, q_c]   (PE, fp16) x2
    P^T = exp(S^T)            (Q pre-scaled by 1/sqrt(D) on host)  (ACT)
      [selected pairs instead use a Schraudolph exp on DVE:
       P = bitcast_fp16(int16(S * 1024*log2e + 15315.5)), max rel err ~3%]
    causal wedge masked via affine_select on the diagonal 128x128s  (GpSimd)
    ctx^T[d, q_c] += matmul: lhsT = V[k_j, d], rhs = P^T           (PE) x2
    pacc[k, q]    += P^T halves                                    (DVE)
  End of chunk: one ones-like one-hot matmul folds pacc into row c of a
  per-head PSUM tile l4[4, 512] (l = softmax denominators); l4 is copied
  out once per head.

Engine balance: PE does only the two real GEMMs + 4 tiny folds/head; the
softmax row-sum accumulation lives on DVE; the causal mask on GpSimd; exp
mostly on ACT with a slice offloaded to DVE.

Host pre-transposes Q/K to [d, s] per head so no on-chip transposes are
needed anywhere, and does the final divide ctx/l (mathematically identical
to normalizing P before the V matmul).
"""

import sys

if "/opt/trn_rl_repo" not in sys.path:
    sys.path.insert(0, "/opt/trn_rl_repo")

import numpy as np

S, B, H, D = 2048, 2, 16, 128
N_CORES = 8
HPC = (B * H) // N_CORES  # head-slices per core = 4
QCH = 512  # query chunk width (one PSUM bank of fp32)
NCH = S // QCH  # 4 chunks
NKB = S // 128  # 16 key blocks
SCALE = 1.0 / float(np.sqrt(D))

QK_DTYPE = "float16"

# Schraudolph fp16 exp: exp(s) ~= bitcast_fp16(int16(s*EXP_A + EXP_B)).
EXP_A = 1024.0 / float(np.log(2.0))
EXP_B = 15315.5

# (c, pi) pairs whose exp runs on DVE instead of ACT (must be
# non-diagonal pairs, i.e. 2*pi+1 < 4*c).
DVE_EXP_PAIRS = {(2, 1), (3, 1)}
# (c, pi) pairs whose pacc accumulation runs on a GpSimd-owned
# accumulator (full-width non-diagonal pairs only).
GPS_PAIRS = set()

_compiled = None


def _build():
    import concourse.tile as tile
    from concourse import bacc, mybir

    f32 = mybir.dt.float32
    i16 = mybir.dt.int16
    qk_dt = getattr(mybir.dt, QK_DTYPE)

    nc = bacc.Bacc("TRN2", target_bir_lowering=False, debug=False)
    qT = nc.dram_tensor("qT", [HPC, D, S], qk_dt, kind="ExternalInput").ap()
    kT = nc.dram_tensor("kT", [HPC, D, S], qk_dt, kind="ExternalInput").ap()
    # v is pre-shuffled on host to [p, j*128+d] (partition-major) so the
    # load is a plain contiguous [128, 512]-per-chunk DMA
    v = nc.dram_tensor("v", [HPC, 128, NKB * 128], qk_dt, kind="ExternalInput").ap()
    out = nc.dram_tensor("out", [HPC, D, S], qk_dt, kind="ExternalOutput").ap()
    lsum = nc.dram_tensor("lsum", [HPC, NCH, QCH], f32, kind="ExternalOutput").ap()

    with tile.TileContext(nc) as tc:
        with (
            tc.tile_pool(name="const", bufs=1) as const_pool,
            tc.tile_pool(name="io", bufs=2) as io_pool,
            tc.tile_pool(name="p", bufs=6) as p_pool,
            tc.tile_pool(name="acc", bufs=2) as acc_pool,
            tc.tile_pool(name="o", bufs=3) as o_pool,
            tc.tile_pool(name="psum_s", bufs=3, space="PSUM") as psum_s,
            tc.tile_pool(name="psum_ctx", bufs=1, space="PSUM") as psum_ctx,
            tc.tile_pool(name="psum_l", bufs=1, space="PSUM") as psum_l,
        ):
            # one-hot stationaries: oh[:, 4c + c] == 1, used to fold chunk
            # c's pacc row-sum into partition c of l4
            oh = const_pool.tile([128, 4 * NCH], qk_dt)
            nc.vector.memset(oh[:], 0.0)
            for c in range(NCH):
                nc.vector.memset(oh[:, 5 * c : 5 * c + 1], 1.0)

            for h in range(HPC):
                # chunked loads so chunk-0 compute starts before the whole
                # head is resident
                qT_s = io_pool.tile([128, S], qk_dt, tag="qT_s")
                kT_s = io_pool.tile([128, S], qk_dt, tag="kT_s")
                v_s = io_pool.tile([128, NKB * 128], qk_dt, tag="v_s")
                for c in range(NCH):
                    sl = slice(c * QCH, (c + 1) * QCH)
                    if h == 0 and c == 0:
                        # split across queues for fast first-compute; issue
                        # order = first-needed order (k0, q all, v, rest of k)
                        nc.sync.dma_start(kT_s[:, 0:128], kT[h][:, 0:128])
                        for q in range(4):
                            s4 = slice(q * 128, (q + 1) * 128)
                            nc.sync.dma_start(qT_s[:, s4], qT[h][:, s4])
                        nc.sync.dma_start(kT_s[:, 128:256], kT[h][:, 128:256])
                        for q in range(2):
                            s4 = slice(q * 256, (q + 1) * 256)
                            nc.sync.dma_start(v_s[:, s4], v[h][:, s4])
                        nc.sync.dma_start(kT_s[:, 256:512], kT[h][:, 256:512])
                    else:
                        nc.sync.dma_start(kT_s[:, sl], kT[h][:, sl])
                        nc.sync.dma_start(qT_s[:, sl], qT[h][:, sl])
                        nc.sync.dma_start(v_s[:, sl], v[h][:, sl])

                chunk_order = (
                    list(range(NCH)) if h == 0 else list(range(NCH - 1, -1, -1))
                )
                l4 = psum_l.tile([NCH, QCH], f32, tag="l4")
                nfolds = 2 * NCH
                fold_idx = 0
                for ci, c in enumerate(chunk_order):
                    qmov = qT_s[:, c * QCH : (c + 1) * QCH]
                    ctx_c = psum_ctx.tile([128, QCH], f32, tag="ctx")
                    pacc = acc_pool.tile([128, 2 * QCH], qk_dt, tag="pacc")
                    npairs = 2 * c + 2
                    # diagonal pairs first: their exp->mask->BMM2 chain then
                    # overlaps the rest of the chunk instead of serializing
                    # at the chunk boundary
                    pair_order = [2 * c, 2 * c + 1] + list(range(2 * c))
                    for pii, pi in enumerate(pair_order):
                        j0, j1 = 2 * pi, 2 * pi + 1
                        off = [j - 4 * c for j in (j0, j1)]
                        # causal trim: for diagonal blocks (off >= 0) the
                        # first 128*off query columns are fully masked;
                        # skip them everywhere.
                        w = [128 * max(0, o) for o in off]
                        s2 = psum_s.tile([128, 2 * QCH], f32, tag="s2")
                        p2 = p_pool.tile([128, 2 * QCH], qk_dt, tag="p2")
                        for o, j in enumerate((j0, j1)):
                            nc.tensor.matmul(
                                s2[:, o * QCH + w[o] : (o + 1) * QCH],
                                kT_s[:, j * 128 : (j + 1) * 128],
                                qmov[:, w[o] :],
                                start=True,
                                stop=True,
                            )
                        if (c, pi) in DVE_EXP_PAIRS:
                            # non-diagonal pair: Schraudolph exp on DVE
                            nc.vector.tensor_scalar(
                                p2[:].bitcast(i16),
                                s2[:],
                                EXP_A,
                                EXP_B,
                                mybir.AluOpType.mult,
                                mybir.AluOpType.add,
                            )
                        elif off[0] == 2:
                            # diagonal pair (offsets 2,3): per-half exp on
                            # the exact surviving spans
                            nc.scalar.activation(
                                p2[:, 256:512],
                                s2[:, 256:512],
                                mybir.ActivationFunctionType.Exp,
                            )
                            nc.scalar.activation(
                                p2[:, QCH + 384 :],
                                s2[:, QCH + 384 :],
                                mybir.ActivationFunctionType.Exp,
                            )
                        else:
                            # non-diagonal pair, or diagonal pair (0,1):
                            # one 1024-wide exp ([512:640] of the (0,1)
                            # pair is stale; the affine_select below
                            # zero-fills it)
                            nc.scalar.activation(
                                p2[:],
                                s2[:],
                                mybir.ActivationFunctionType.Exp,
                            )
                        # causal wedge masks (keep where x' - p >= 0)
                        if off[0] == 0:
                            # both blocks' wedges (plus the h1 stale prefix
                            # [512:640]) in ONE select over a [p, o, x] view:
                            # keep iff x - 128*o - p >= 0; cols x in
                            # [256:512) of either half are always kept and
                            # excluded from the AP
                            sel = p2[:].rearrange("p (o x) -> p o x", o=2)[
                                :, :, 0:256
                            ]
                            nc.gpsimd.affine_select(
                                sel,
                                sel,
                                pattern=[[-128, 2], [1, 256]],
                                base=0,
                                channel_multiplier=-1,
                                compare_op=mybir.AluOpType.is_ge,
                                fill=0.0,
                            )
                        elif off[0] == 2:
                            nc.gpsimd.affine_select(
                                p2[:, 256:384],
                                p2[:, 256:384],
                                pattern=[[1, 128]],
                                base=0,
                                channel_multiplier=-1,
                                compare_op=mybir.AluOpType.is_ge,
                                fill=0.0,
                            )
                            nc.gpsimd.affine_select(
                                p2[:, QCH + 384 :],
                                p2[:, QCH + 384 :],
                                pattern=[[1, 128]],
                                base=0,
                                channel_multiplier=-1,
                                compare_op=mybir.AluOpType.is_ge,
                                fill=0.0,
                            )
                        for o, j in enumerate((j0, j1)):
                            nc.tensor.matmul(
                                ctx_c[:, w[o] :],
                                v_s[:, j * 128 : (j + 1) * 128],
                                p2[:, o * QCH + w[o] : (o + 1) * QCH],
                                start=(pii == 0 and o == 0),
                                stop=(pii == npairs - 1 and o == 1),
                                skip_group_check=True,
                            )
                        # accumulate P into pacc / paccg: pair (0,1)
                        # initializes pacc with a full copy (its masked
                        # regions are zero after the select); pair (2,3)
                        # adds only its surviving spans; non-diagonal pairs
                        # add full 1024-wide (one DVE op per pair), except
                        # GPS_PAIRS which accumulate on GpSimd
                        if pii == 0:
                            nc.vector.tensor_copy(pacc[:], p2[:])
                        elif off[0] == 2:
                            nc.vector.tensor_add(
                                pacc[:, 256:512],
                                pacc[:, 256:512],
                                p2[:, 256:512],
                            )
                            nc.vector.tensor_add(
                                pacc[:, QCH + 384 :],
                                pacc[:, QCH + 384 :],
                                p2[:, QCH + 384 :],
                            )
                        else:
                            # 512-wide halves: fp16 tensor_tensor only gets
                            # the 2x uop up to FD=512; 1024-wide runs 1x
                            nc.vector.tensor_add(
                                pacc[:, 0:QCH], pacc[:, 0:QCH], p2[:, 0:QCH]
                            )
                            nc.vector.tensor_add(
                                pacc[:, QCH:], pacc[:, QCH:], p2[:, QCH:]
                            )
                    # fold chunk row-sums into partition c of l4 (both
                    # halves of each accumulator)
                    for fsrc in (pacc[:, 0:QCH], pacc[:, QCH:]):
                        nc.tensor.matmul(
                            l4[:],
                            oh[:, 4 * c : 4 * c + 4],
                            fsrc,
                            start=(fold_idx == 0),
                            stop=(fold_idx == nfolds - 1),
                            skip_group_check=True,
                        )
                        fold_idx += 1
                    o_t = o_pool.tile([128, QCH], qk_dt, tag="o")
                    # drain ctx in two halves on different engines: spreads
                    # load and frees the PSUM bank sooner
                    nc.scalar.copy(o_t[:, 0:256], ctx_c[:, 0:256])
                    nc.vector.tensor_copy(o_t[:, 256:], ctx_c[:, 256:])
                    nc.sync.dma_start(
                        out[h][:, c * QCH : (c + 1) * QCH], o_t[:]
                    )
                lo_t = o_pool.tile([NCH, QCH], f32, tag="lo")
                nc.vector.tensor_copy(lo_t[:], l4[:])
                nc.sync.dma_start(lsum[h], lo_t[:])

    nc.compile()
    return nc


def _get_compiled():
    global _compiled
    if _compiled is None:
        _compiled = _build()
    return _compiled


def _run(query_layer, key_layer, value_layer, attention_mask=None, trace=False):
    from concourse import bass_utils

    nc = _get_compiled()

    q = np.asarray(query_layer, dtype=np.float32)
    k = np.asarray(key_layer, dtype=np.float32)
    v = np.asarray(value_layer, dtype=np.float32)

    np_dt = np.float16 if QK_DTYPE == "float16" else np.float32

    # [S,B,H,D] -> [BH, D, S] for q/k, [BH, S, D] for v.
    # Fold the 1/sqrt(D) softmax scale into Q on the host.
    qT_all = np.ascontiguousarray(
        (q.transpose(1, 2, 3, 0).reshape(B * H, D, S) * np.float32(SCALE)).astype(
            np_dt
        )
    )
    kT_all = np.ascontiguousarray(
        k.transpose(1, 2, 3, 0).reshape(B * H, D, S).astype(np_dt)
    )
    # [S,B,H,D] -> [BH, S, D] -> partition-major [BH, p, j, d] with
    # s = 128*j + p, flattened to [BH, 128, NKB*128]
    v_all = np.ascontiguousarray(
        v.transpose(1, 2, 0, 3)
        .reshape(B * H, NKB, 128, D)
        .transpose(0, 2, 1, 3)
        .reshape(B * H, 128, NKB * 128)
        .astype(np_dt)
    )

    in_maps = [
        {
            "qT": qT_all[c * HPC : (c + 1) * HPC],
            "kT": kT_all[c * HPC : (c + 1) * HPC],
            "v": v_all[c * HPC : (c + 1) * HPC],
        }
        for c in range(N_CORES)
    ]
    res = bass_utils.run_bass_kernel_spmd(
        nc, in_maps, list(range(N_CORES)), trace=trace
    )

    ctxT = np.concatenate(
        [np.asarray(res.results[c]["out"], dtype=np.float32) for c in range(N_CORES)],
        axis=0,
    )  # [BH, D, S]
    l = np.concatenate(
        [res.results[c]["lsum"].reshape(HPC, S) for c in range(N_CORES)], axis=0
    )  # [BH, S]
    ctxT = ctxT / l[:, None, :]
    # [BH, D, S] -> [S, B, H*D]
    full = ctxT.reshape(B, H, D, S).transpose(3, 0, 1, 2).reshape(S, B, H * D)
    return np.ascontiguousarray(full.astype(np.float32)), res


def kernel(query_layer, key_layer, value_layer, attention_mask=None):
    out, _ = _run(query_layer, key_layer, value_layer, attention_mask)
    return out
